# revision 35
# baseline (speedup 1.0000x reference)
"""Trainium2 Bass kernel for nn_BaselineGPT (sliding-window GQA attention block).

Sharding: 8 cores = 2 batches x 4 sequence chunks of 512 queries.
Each core computes its 512 output rows end-to-end (QKV proj, RMS norm, RoPE,
windowed GQA attention, output proj).  KV halo of 256 rows comes with the
chunk; chunk 0's missing halo is masked via a -30000 additive score bias
folded into an extra row of K^T.  Pair-head mixing is folded into Wo on the
host (it is linear and applied post-normalization).
"""

import math
from contextlib import ExitStack

import numpy as np

import concourse.bass as bass
from concourse import bacc
import concourse.mybir as mybir
import concourse.tile as tile
from concourse.masks import make_identity

B, S, DIM = 2, 2048, 1024
H, KVH, HD = 16, 4, 64
WINDOW = 256
ROPE_BASE = 10000.0
EPS = 1e-6

NQ = 512          # queries per core
NK = 768          # kv rows per core (incl 256 halo)
NCORES = 8
F32 = mybir.dt.float32
F32R = mybir.dt.float32r
BF16 = mybir.dt.bfloat16

_BUILT = None


def _build():
    nc = bacc.Bacc(None)

    xt = nc.declare_dram_parameter("xt", [DIM, NK], BF16, isOutput=False)
    wq = nc.declare_dram_parameter("wq", [DIM, DIM], BF16, isOutput=False)
    wk = nc.declare_dram_parameter("wk", [DIM, KVH * HD], BF16, isOutput=False)
    wv = nc.declare_dram_parameter("wv", [DIM, KVH * HD], BF16, isOutput=False)
    wo = nc.declare_dram_parameter("wo", [DIM, DIM], BF16, isOutput=False)
    cosk = nc.declare_dram_parameter("cosk", [NK, HD // 2], F32, isOutput=False)
    sink = nc.declare_dram_parameter("sink", [NK, HD // 2], F32, isOutput=False)
    kbias = nc.declare_dram_parameter("kbias", [1, NK], BF16, isOutput=False)
    qgain = nc.declare_dram_parameter("qgain", [1, H], F32, isOutput=False)
    m0 = nc.declare_dram_parameter("m0", [128, 512], BF16, isOutput=False)
    m2 = nc.declare_dram_parameter("m2", [128, 512], BF16, isOutput=False)
    out = nc.declare_dram_parameter("out", [NQ, DIM], F32, isOutput=True)

    with tile.TileContext(nc) as tc, ExitStack() as ctx:
        const = ctx.enter_context(tc.tile_pool(name="const", bufs=1))
        big = ctx.enter_context(tc.tile_pool(name="big", bufs=1))
        tmp = ctx.enter_context(tc.tile_pool(name="tmp", bufs=3))
        qtg_pool = ctx.enter_context(tc.tile_pool(name="qtg", bufs=5))
        att_pool = ctx.enter_context(tc.tile_pool(name="att", bufs=4))
        tn_pool = ctx.enter_context(tc.tile_pool(name="tn", bufs=2))
        outp = ctx.enter_context(tc.tile_pool(name="outp", bufs=2))
        ps_pool = ctx.enter_context(tc.tile_pool(name="ps", bufs=6, space="PSUM"))
        ps_bt = ctx.enter_context(tc.tile_pool(name="psbt", bufs=2, space="PSUM"))

        # ---- constants / small inputs ----
        ident = const.tile([128, 128], BF16, tag="ident")
        make_identity(nc, ident)
        ident_f32 = const.tile([128, 128], F32, tag="identf")
        make_identity(nc, ident_f32)
        eps_t = const.tile([128, 1], F32, tag="eps")
        nc.vector.memset(eps_t, EPS)
        ones64 = const.tile([1, 64], BF16, tag="ones64")
        nc.vector.memset(ones64, 1.0)
        qg_sb = const.tile([128, H], F32, tag="qg")
        nc.sync.dma_start(out=qg_sb, in_=qgain[0:1, :].to_broadcast((128, H)))
        m0_sb = const.tile([128, 512], BF16, tag="m0")
        nc.sync.dma_start(out=m0_sb, in_=m0[:, :])
        m2_sb = const.tile([128, 512], BF16, tag="m2")
        nc.sync.dma_start(out=m2_sb, in_=m2[:, :])
        cos_sb, sin_sb = [], []
        for st in range(6):
            sl = slice(st * 128, st * 128 + 128)
            tc_ = const.tile([128, HD // 2], F32, tag=f"cos{st}")
            nc.sync.dma_start(out=tc_, in_=cosk[sl, :])
            cos_sb.append(tc_)
            ts_ = const.tile([128, HD // 2], F32, tag=f"sin{st}")
            nc.sync.dma_start(out=ts_, in_=sink[sl, :])
            sin_sb.append(ts_)

        # ---- big persistent SBUF tensors ----
        xt_sb, wq_sb, wkv_sb = [], [], []
        for kt_ in range(8):
            sl = slice(kt_ * 128, kt_ * 128 + 128)
            for lst, nm, dram, w in (
                (xt_sb, "xt", xt, NK),
                (wq_sb, "wq", wq, DIM),
            ):
                t = big.tile([128, w], BF16, tag=f"{nm}{kt_}", name=f"{nm}{kt_}")
                nc.sync.dma_start(out=t, in_=dram[sl, :])
                lst.append(t)
            t = big.tile([128, 512], BF16, tag=f"wkv{kt_}", name=f"wkv{kt_}")
            nc.scalar.dma_start(out=t[:, 0 : KVH * HD], in_=wk[sl, :])
            nc.scalar.dma_start(out=t[:, KVH * HD :], in_=wv[sl, :])
            wkv_sb.append(t)
        q_rope = big.tile([128, 4, DIM], BF16, tag="qrope")
        k_rope = big.tile([128, 6, KVH * HD], BF16, tag="krope")
        v_sb = big.tile([128, 6, KVH, HD + 1], BF16, tag="v")
        kt_sb = big.tile([128, KVH, NK], BF16, tag="kt")
        yraw = big.tile([64, 16, 512], BF16, tag="yraw")
        den_all = big.tile([16, 512], F32, tag="denall")
        rec_all = big.tile([16, 512], BF16, tag="recall")

        nc.vector.memset(v_sb[:, :, :, HD : HD + 1], 1.0)
        # bias row (row 64) of each k^T block, staged via SBUF to keep
        # kt_sb's writers on compute engines only (one semaphore)
        kb_sb = const.tile([1, NK], BF16, tag="kb")
        nc.sync.dma_start(out=kb_sb, in_=kbias[:, :])
        for g in range(KVH):
            nc.vector.tensor_copy(out=kt_sb[64:65, g, :], in_=kb_sb)

        def rmsnorm_rope(src_psum, nheads, st, dst, gain):
            """src_psum [128, nheads*HD] -> dst (slice of *_rope) with RMS norm,
            optional per-head gain (incl 1/8 scaling), and RoPE at kv tile st."""
            hw = nheads * HD
            sq = tmp.tile([128, 16, HD], F32, tag="sq")
            nc.scalar.activation(
                out=sq[:, :nheads, :],
                in_=src_psum.rearrange("p (h d) -> p h d", d=HD),
                func=mybir.ActivationFunctionType.Square,
            )
            ssq = tmp.tile([128, 16], F32, tag="ssq")
            nc.vector.tensor_reduce(
                out=ssq[:, :nheads],
                in_=sq[:, :nheads, :],
                axis=mybir.AxisListType.X,
                op=mybir.AluOpType.add,
            )
            # sqrt(mean + eps) then reciprocal (Rsqrt activation is banned)
            nc.scalar.activation(
                out=ssq[:, :nheads],
                in_=ssq[:, :nheads],
                func=mybir.ActivationFunctionType.Sqrt,
                bias=eps_t,
                scale=1.0 / HD,
            )
            inv = tmp.tile([128, 16], F32, tag="inv")
            nc.vector.reciprocal(out=inv[:, :nheads], in_=ssq[:, :nheads])
            if gain:
                nc.vector.tensor_mul(
                    out=inv[:, :nheads], in0=inv[:, :nheads], in1=qg_sb[:, :nheads]
                )
            invf = tmp.tile([128, 16, HD], F32, tag="invf")
            nc.vector.tensor_copy(
                out=invf[:, :nheads, :],
                in_=inv[:, :nheads].rearrange("p (h o) -> p h o", o=1).broadcast_to(
                    (128, nheads, HD)
                ),
            )
            rn = tmp.tile([128, 16, HD], F32, tag="rn")
            nc.vector.tensor_mul(
                out=rn[:, :nheads, :],
                in0=src_psum.rearrange("p (h d) -> p h d", d=HD),
                in1=invf[:, :nheads, :],
            )
            # RoPE: out1 = r1*cos + r2*sin ; out2 = r2*cos - r1*sin
            hd2 = HD // 2
            r1 = rn[:, :nheads, 0:hd2]
            r2 = rn[:, :nheads, hd2:HD]
            cosb = cos_sb[st].rearrange("p (o f) -> p o f", o=1).broadcast_to(
                (128, nheads, hd2)
            )
            sinb = sin_sb[st].rearrange("p (o f) -> p o f", o=1).broadcast_to(
                (128, nheads, hd2)
            )
            dd = dst.rearrange("p (h d) -> p h d", d=HD)
            o1 = dd[:, :, 0:hd2]
            o2 = dd[:, :, hd2:HD]
            t1 = tmp.tile([128, 16, hd2], F32, tag="ropet1")
            t2 = tmp.tile([128, 16, hd2], F32, tag="ropet2")
            nc.vector.tensor_mul(out=t1[:, :nheads, :], in0=r1, in1=cosb)
            nc.vector.tensor_mul(out=t2[:, :nheads, :], in0=r2, in1=sinb)
            nc.vector.tensor_add(out=o1, in0=t1[:, :nheads, :], in1=t2[:, :nheads, :])
            nc.vector.tensor_mul(out=t1[:, :nheads, :], in0=r2, in1=cosb)
            nc.vector.tensor_mul(out=t2[:, :nheads, :], in0=r1, in1=sinb)
            nc.vector.tensor_sub(out=o2, in0=t1[:, :nheads, :], in1=t2[:, :nheads, :])

        # ---- fused K|V projection over 6 kv s-tiles ----
        for st in range(6):
            pkv = ps_pool.tile([128, 512], F32, tag="p512")
            for kt_ in range(8):
                nc.tensor.matmul(
                    out=pkv,
                    lhsT=xt_sb[kt_][:, st * 128 : st * 128 + 128],
                    rhs=wkv_sb[kt_],
                    start=(kt_ == 0),
                    stop=(kt_ == 7),
                )
            nc.vector.tensor_copy(
                out=v_sb[:, st, :, 0:HD],
                in_=pkv[:, KVH * HD :].rearrange("p (g d) -> p g d", d=HD),
            )
            rmsnorm_rope(pkv[:, 0 : KVH * HD], KVH, st, k_rope[:, st, :], gain=False)

        # ---- Q projection over 4 q s-tiles (kv rows 256..768) ----
        for st in range(4):
            for half in range(2):
                pq = ps_pool.tile([128, 512], F32, tag="p512")
                for kt_ in range(8):
                    nc.tensor.matmul(
                        out=pq,
                        lhsT=xt_sb[kt_][:, 256 + st * 128 : 384 + st * 128],
                        rhs=wq_sb[kt_][:, half * 512 : half * 512 + 512],
                        start=(kt_ == 0),
                        stop=(kt_ == 7),
                    )
                rmsnorm_rope(
                    pq, 8, st + 2, q_rope[:, st, half * 512 : half * 512 + 512],
                    gain=True,
                )

        # wo reuses xt's SBUF slot (xt's last use is the Q projection above)
        wo_sb = []
        for h in range(H):
            t = big.tile([64, DIM], BF16, tag=f"wo{h}", name=f"wo{h}")
            nc.scalar.dma_start(out=t, in_=wo[h * 64 : h * 64 + 64, :])
            wo_sb.append(t)

        # ---- transpose K: k_rope [128s, (g,d)] -> kt_sb [d, g, s] ----
        for g in range(KVH):
            for half in range(2):
                ptk = ps_bt.tile([128, 512], BF16, tag="p512b")
                for i in range(3):
                    st = half * 3 + i
                    nc.tensor.transpose(
                        out=ptk[0:64, i * 128 : i * 128 + 128],
                        in_=k_rope[:, st, g * HD : g * HD + HD],
                        identity=ident,
                    )
                nc.vector.tensor_copy(
                    out=kt_sb[0:64, g, half * 384 : half * 384 + 384],
                    in_=ptk[0:64, 0:384],
                )

        # ---- per group: transpose Q tiles then attention over qblocks ----
        for g in range(KVH):
            qtg = {}
            for st in range(4):
                ptq = ps_bt.tile([128, 512], BF16, tag="p512b")
                for hh in range(4):
                    h = g * 4 + hh
                    nc.tensor.transpose(
                        out=ptq[0:64, hh * 128 : hh * 128 + 128],
                        in_=q_rope[:, st, h * HD : h * HD + HD],
                        identity=ident,
                    )
                qt = qtg_pool.tile([128, 512], BF16, tag="qtg")
                nc.vector.tensor_copy(out=qt[0:64, :], in_=ptq[0:64, :])
                nc.vector.memset(qt[64:65, :], 1.0)
                qtg[(g, st)] = qt

            for qb in range(4):
                att = att_pool.tile([128, 1536], BF16, tag="att")
                for t in range(3):
                    pss = ps_pool.tile([128, 512], F32, tag="p512")
                    nc.tensor.matmul(
                        out=pss,
                        lhsT=kt_sb[
                            0:65, g, qb * 128 + t * 128 : qb * 128 + t * 128 + 128
                        ],
                        rhs=qtg[(g, qb)][0:65, :],
                        start=True,
                        stop=True,
                    )
                    nc.scalar.activation(
                        out=att[:, t * 512 : t * 512 + 512],
                        in_=pss,
                        func=mybir.ActivationFunctionType.Exp,
                    )
                nc.vector.tensor_mul(out=att[:, 0:512], in0=att[:, 0:512], in1=m0_sb)
                nc.vector.tensor_mul(
                    out=att[:, 1024:1536], in0=att[:, 1024:1536], in1=m2_sb
                )
                psy = ps_pool.tile([128, 512], F32, tag="p512")
                for t in range(3):
                    nc.tensor.matmul(
                        out=psy[0:65, :],
                        lhsT=v_sb[:, qb + t, g, :],
                        rhs=att[:, t * 512 : t * 512 + 512],
                        start=(t == 0),
                        stop=(t == 2),
                    )
                it = g * 4 + qb
                nc.scalar.copy(out=yraw[:, it, :], in_=psy[0:64, :])
                dr = tn_pool.tile([1, 512], F32, tag="dr")
                nc.scalar.copy(out=dr, in_=psy[64:65, :])
                nc.sync.dma_start(out=den_all[it : it + 1, :], in_=dr)

        # ---- batched softmax reciprocal: [16,512] -> [128,64] -> recip ----
        pden = ps_pool.tile([128, 512], F32, tag="p512")
        for c in range(4):
            nc.tensor.transpose(
                out=pden[:, c * 16 : c * 16 + 16],
                in_=den_all[:, c * 128 : c * 128 + 128],
                identity=ident_f32[0:16, 0:16],
            )
        rc = tn_pool.tile([128, 64], F32, tag="rc")
        nc.vector.reciprocal(out=rc, in_=pden[:, 0:64])
        prow = ps_pool.tile([128, 512], F32, tag="p512")
        for c in range(4):
            nc.tensor.transpose(
                out=prow[0:16, c * 128 : c * 128 + 128],
                in_=rc[:, c * 16 : c * 16 + 16],
                identity=ident_f32,
            )
        nc.scalar.copy(out=rec_all, in_=prow[0:16, :])

        # ---- normalize yraw in place, then project per qblock ----
        for qb in range(4):
            for g in range(KVH):
                it = g * 4 + qb
                rrow = tn_pool.tile([1, 512], BF16, tag="rrow")
                nc.sync.dma_start(out=rrow, in_=rec_all[it : it + 1, :])
                rb = ps_pool.tile([128, 512], F32, tag="p512")
                nc.tensor.matmul(
                    out=rb[0:64, :], lhsT=ones64, rhs=rrow, start=True, stop=True
                )
                nc.vector.tensor_mul(
                    out=yraw[:, it, :], in0=yraw[:, it, :], in1=rb[0:64, :]
                )
            ob = outp.tile([128, DIM], F32, tag="ob")
            for half in range(2):
                po = ps_pool.tile([128, 512], F32, tag="p512")
                for h in range(H):
                    g, hh = h // 4, h % 4
                    nc.tensor.matmul(
                        out=po,
                        lhsT=yraw[:, g * 4 + qb, hh * 128 : hh * 128 + 128],
                        rhs=wo_sb[h][:, half * 512 : half * 512 + 512],
                        start=(h == 0),
                        stop=(h == H - 1),
                    )
                nc.scalar.copy(out=ob[:, half * 512 : half * 512 + 512], in_=po)
            nc.gpsimd.dma_start(out=out[qb * 128 : qb * 128 + 128, :], in_=ob)

    nc.finalize()
    return nc


def _host_inputs(x, Wq, Wk, Wv, Wo, q_gain, pair_mix):
    """Build the 8 per-core input maps."""
    x = np.asarray(x, np.float32)
    Wq = np.asarray(Wq, np.float32)
    Wk = np.asarray(Wk, np.float32)
    Wv = np.asarray(Wv, np.float32)
    Wo = np.asarray(Wo, np.float32)
    q_gain = np.asarray(q_gain, np.float32)
    pair_mix = np.asarray(pair_mix, np.float32)

    # fold pair mixing into Wo:  out = y_mix @ Wo.T,  y_mix = y @ M.T  =>  Wo' = Wo @ M
    M = np.zeros((DIM, DIM), np.float32)
    eye = np.eye(HD, dtype=np.float32)
    for p in range(H // 2):
        for o in range(2):
            for i in range(2):
                ho, hi = 2 * p + o, 2 * p + i
                M[ho * HD : ho * HD + HD, hi * HD : hi * HD + HD] = (
                    pair_mix[p, o, i] * eye
                )
    woT = np.ascontiguousarray((Wo @ M).T)

    wqT = np.ascontiguousarray(Wq.T)
    wkT = np.ascontiguousarray(Wk.T)
    wvT = np.ascontiguousarray(Wv.T)
    qg8 = (q_gain / math.sqrt(HD)).reshape(1, H).astype(np.float32)

    inv_freq = 1.0 / (ROPE_BASE ** (np.arange(0, HD, 2, dtype=np.float32) / HD))

    ql = np.arange(128)
    m0_ = (ql[:, None] >= ql[None, :] + 1).astype(np.float32)  # kl >= ql+1
    m2_ = (ql[:, None] <= ql[None, :]).astype(np.float32)      # kl <= ql
    m0t = np.ascontiguousarray(np.tile(m0_, (1, 4)))
    m2t = np.ascontiguousarray(np.tile(m2_, (1, 4)))

    import ml_dtypes
    bf = ml_dtypes.bfloat16
    wqT, wkT, wvT, woT = (a.astype(bf) for a in (wqT, wkT, wvT, woT))
    m0t, m2t = m0t.astype(bf), m2t.astype(bf)
    in_maps = []
    for core in range(NCORES):
        b, c = core // 4, core % 4
        ks = 512 * c - 256
        xc = np.zeros((NK, DIM), np.float32)
        lo = max(0, ks)
        xc[lo - ks :] = x[b, lo : ks + NK]
        t = (ks + np.arange(NK, dtype=np.float32))[:, None]
        freqs = t * inv_freq[None, :]
        kb = np.where(t[:, 0] < 0, -30000.0, 0.0).astype(np.float32).reshape(1, NK)
        in_maps.append(
            {
                "xt": np.ascontiguousarray(xc.T).astype(bf),
                "wq": wqT,
                "wk": wkT,
                "wv": wvT,
                "wo": woT,
                "cosk": np.cos(freqs).astype(np.float32),
                "sink": np.sin(freqs).astype(np.float32),
                "kbias": kb.astype(bf),
                "qgain": qg8,
                "m0": m0t,
                "m2": m2t,
            }
        )
    return in_maps


def kernel(x, Wq, Wk, Wv, Wo, q_gain, pair_mix):
    global _BUILT
    from concourse.bass_utils import run_bass_kernel_spmd

    if _BUILT is None:
        _BUILT = _build()
    in_maps = _host_inputs(x, Wq, Wk, Wv, Wo, q_gain, pair_mix)
    res = run_bass_kernel_spmd(_BUILT, in_maps, list(range(NCORES)))
    out = np.empty((B, S, DIM), np.float32)
    for core in range(NCORES):
        b, c = core // 4, core % 4
        out[b, 512 * c : 512 * c + 512, :] = res.results[core]["out"]
    return out
